# revision 15
# baseline (speedup 1.0000x reference)
"""Trainium2 Bass kernel for nn_MixedHOMVector (higher-order moment pooling).

Reference computation (per batch row b, channel c, pooling over T):
    grp  = mean(x**p)            (p scalar; p==1 -> grp == mean(x))
    mu   = mean(x); var = mean((x-mu)^2)
    skew = mean((x-mu)^3) / (var+EPS)^1.5
    kurt = mean((x-mu)^4) / (var+EPS)^2
    out  = concat([grp, var, skew, kurt], -1)    # [B, 4C]

Strategy (data-parallel over batch, 8 cores, B/8 = 4 rows each):
  * Layout: tiles [128 part = t-sub, free = (s, c)]; each DMA moves a
    contiguous 1 MiB block of x.
  * Shifted power sums: y = x - 0.5. Compute y, y2, y3, y4 element-wise
    (bf16 outputs) split across DVE/ACT, then sum over t with TensorE
    ones-matmuls accumulating in PSUM (bf16 rhs = 1 col/cycle).
    Four moments go to four PE column groups (tile_position col packing)
    so their matmuls run concurrently.
  * Central moments recovered from shifted raw sums in a tiny fp32
    epilogue done in a [c-partition, moment-free] transposed layout.
"""

import sys

if "/opt/trn_rl_repo" not in sys.path:
    sys.path.insert(0, "/opt/trn_rl_repo")

import numpy as np

B, T, C = 32, 8192, 256
N_CORES = 8
B_LOC = B // N_CORES          # batch rows per core
EPS = 1e-6
SHIFT = 0.5                   # constant shift for numerical stability

P = 128                       # SBUF partitions
TT = 1024                     # t-rows per big tile (1 MiB f32 per tile)
S = TT // P                   # t-rows per partition within a tile
FREE = S * C                  # free elements per partition per tile (2048)
NTILES = T // TT              # big tiles per batch row (8)
MMN = 512                     # matmul moving free dim (max for one PSUM bank)
NSLICE = FREE // MMN          # matmul slices per tile (4)

_CACHE = {}


def _build(p_val: float, repeat: int = 1):
    """Build + compile the per-core SPMD bass kernel. p_val==1.0 uses the
    fast path (grp == mean); otherwise an extra x**p = exp(p*ln x) pass.

    repeat>1 re-runs the main reduction loop (for timing-by-slope: the
    wall-clock difference between repeat=R2 and R1 isolates device time).
    Each repetition restarts PSUM accumulation, so the result is identical."""
    import concourse.bass as bass  # noqa: F401
    import concourse.tile as tile
    from concourse import bacc, mybir
    from contextlib import ExitStack

    f32 = mybir.dt.float32
    bf16 = mybir.dt.bfloat16
    A = mybir.ActivationFunctionType
    OP = mybir.AluOpType

    p_is_one = (p_val == 1.0)
    NMOM = 4 if p_is_one else 5

    nc = bacc.Bacc("TRN2", target_bir_lowering=False, debug=False,
                   num_devices=N_CORES)

    x = nc.dram_tensor("x", [B_LOC, T, C], f32, kind="ExternalInput").ap()
    out = nc.dram_tensor("out", [B_LOC, 4 * C], f32, kind="ExternalOutput").ap()
    scratch = nc.dram_tensor("scratch", [B_LOC, NMOM, MMN], f32).ap()

    # [B_LOC, NTILES, P, (s c)] view of x; per (b, j) a contiguous 1MiB block
    xv = x.rearrange("b (n p s) c -> b n p (s c)", p=P, s=S)

    with tile.TileContext(nc) as tc, ExitStack() as ctx:
        xp = ctx.enter_context(tc.tile_pool(name="xp", bufs=3))
        yp = ctx.enter_context(tc.tile_pool(name="yp", bufs=2))
        pp = ctx.enter_context(tc.tile_pool(name="pp", bufs=4, space="PSUM"))
        sp = ctx.enter_context(tc.tile_pool(name="sp", bufs=1))
        ep = ctx.enter_context(tc.tile_pool(name="ep", bufs=1))

        ones = sp.tile([P, 1], bf16)
        nc.vector.memset(ones, 1.0)
        neg_shift = sp.tile([P, 1], f32)
        nc.vector.memset(neg_shift, -SHIFT)
        zero_b = sp.tile([P, 1], f32)
        nc.vector.memset(zero_b, 0.0)

        # stage rows live at partitions 32*m (moment m), free = (b, s2, c)
        stage = ep.tile([P, B_LOC * MMN], f32)
        stage5 = ep.tile([1, B_LOC * MMN], f32) if not p_is_one else None

        for rep, b in [(r, b) for r in range(repeat) for b in range(B_LOC)]:
            psum = pp.tile([P, MMN], f32, tag="psum")     # 1 bank; rows 0/32/64/96
            psum5 = pp.tile([1, MMN], f32, tag="psum5") if not p_is_one else None
            for j in range(NTILES):
                xt = xp.tile([P, FREE], f32, tag="xt")
                nc.sync.dma_start(out=xt[:], in_=xv[b, j])

                y1 = yp.tile([P, FREE], bf16, tag="y1")
                nc.vector.tensor_scalar_add(y1[:], xt[:], -SHIFT)

                y2 = yp.tile([P, FREE], bf16, tag="y2")
                nc.scalar.activation(y2[:], xt[:], A.Square,
                                     bias=neg_shift[:], scale=1.0)

                y3 = yp.tile([P, FREE], bf16, tag="y3")
                nc.vector.tensor_mul(y3[:], y1[:], y2[:])

                y4 = yp.tile([P, FREE], bf16, tag="y4")
                if j % 2 == 0:
                    nc.scalar.activation(y4[:], y2[:], A.Square,
                                         bias=zero_b[:], scale=1.0)
                else:
                    nc.vector.tensor_mul(y4[:], y2[:], y2[:])

                moms = [y1, y2, y3, y4]
                if not p_is_one:
                    lnx = yp.tile([P, FREE], f32, tag="lnx")
                    nc.scalar.activation(lnx[:], xt[:], A.Log,
                                         bias=zero_b[:], scale=1.0)
                    xpw = yp.tile([P, FREE], bf16, tag="xpw")
                    nc.scalar.activation(xpw[:], lnx[:], A.Exp,
                                         bias=zero_b[:], scale=p_val)

                for k in range(NSLICE):
                    first = (j == 0 and k == 0)
                    last = (j == NTILES - 1 and k == NSLICE - 1)
                    for m, ym in enumerate(moms):
                        nc.tensor.matmul(
                            psum[32 * m: 32 * m + 1, :],
                            ones[:],
                            ym[:, k * MMN: (k + 1) * MMN],
                            start=first, stop=last,
                            tile_position=(0, 32 * m),
                        )
                    if not p_is_one:
                        nc.tensor.matmul(
                            psum5[:], ones[:],
                            xpw[:, k * MMN: (k + 1) * MMN],
                            start=first, stop=last,
                        )

            for m in range(4):
                nc.scalar.copy(stage[32 * m: 32 * m + 1, b * MMN: (b + 1) * MMN],
                               psum[32 * m: 32 * m + 1, :])
            if not p_is_one:
                nc.scalar.copy(stage5[:, b * MMN: (b + 1) * MMN], psum5[:])

        # stage -> scratch[b, m, (s2 c)]
        for m in range(4):
            nc.sync.dma_start(
                out=scratch[:, m, :],
                in_=stage[32 * m: 32 * m + 1, :].rearrange(
                    "p (b f) -> p b f", b=B_LOC),
            )
        if not p_is_one:
            nc.sync.dma_start(
                out=scratch[:, 4, :],
                in_=stage5[:].rearrange("p (b f) -> p b f", b=B_LOC))

        # transposed load-back: [c%128, (s2, h, b, m)]
        mom2 = ep.tile([P, 2 * 2 * B_LOC * NMOM], f32)
        scr_v = scratch.rearrange("b m (s2 h p) -> s2 h p b m", s2=2, h=2, p=P)
        mom2_v = mom2[:].rearrange("p (s2 h b m) -> p s2 h b m",
                                   s2=2, h=2, b=B_LOC, m=NMOM)
        for s2 in range(2):
            for h in range(2):
                nc.sync.dma_start(out=mom2_v[:, s2, h], in_=scr_v[s2, h])
        # fold s-parity: mom[p, (h, b, m)]
        NF = 2 * B_LOC * NMOM
        mom = ep.tile([P, NF], f32)
        nc.vector.tensor_add(mom[:], mom2[:, 0:NF], mom2[:, NF:2 * NF])

        # per-moment [P, (h, b)] views
        momv = mom[:].rearrange("p (h b m) -> p m (h b)", h=2, b=B_LOC, m=NMOM)
        invT = 1.0 / T
        G = 2 * B_LOC  # free size of one moment slice

        def et(name):
            return ep.tile([P, G], f32, name=name)

        d, e2, e3, e4 = et("d"), et("e2"), et("e3"), et("e4")
        nc.vector.tensor_scalar_mul(d[:], momv[:, 0], invT)   # mean(y) = mu-0.5
        nc.vector.tensor_scalar_mul(e2[:], momv[:, 1], invT)
        nc.vector.tensor_scalar_mul(e3[:], momv[:, 2], invT)
        nc.vector.tensor_scalar_mul(e4[:], momv[:, 3], invT)

        # feat[p, (h, b, f)]  f = (grp, var, skew, kurt)
        feat = ep.tile([P, 2 * B_LOC * 4], f32)
        featv = feat[:].rearrange("p (h b f) -> p f (h b)", h=2, b=B_LOC, f=4)

        if p_is_one:
            nc.vector.tensor_scalar_add(featv[:, 0], d[:], SHIFT)   # grp = mu
        else:
            nc.vector.tensor_scalar_mul(featv[:, 0], momv[:, 4], invT)

        d2 = et("d2")
        nc.vector.tensor_mul(d2[:], d[:], d[:])
        nc.vector.tensor_sub(featv[:, 1], e2[:], d2[:])             # var
        d3, t1, m3 = et("d3"), et("t1"), et("m3")
        nc.vector.tensor_mul(d3[:], d2[:], d[:])
        nc.vector.tensor_mul(t1[:], d[:], e2[:])
        nc.vector.scalar_tensor_tensor(m3[:], t1[:], -3.0, e3[:], OP.mult, OP.add)
        nc.vector.scalar_tensor_tensor(m3[:], d3[:], 2.0, m3[:], OP.mult, OP.add)
        t2, t3, d4, m4 = et("t2"), et("t3"), et("d4"), et("m4")
        nc.vector.tensor_mul(t2[:], d[:], e3[:])
        nc.vector.scalar_tensor_tensor(m4[:], t2[:], -4.0, e4[:], OP.mult, OP.add)
        nc.vector.tensor_mul(t3[:], d2[:], e2[:])
        nc.vector.scalar_tensor_tensor(m4[:], t3[:], 6.0, m4[:], OP.mult, OP.add)
        nc.vector.tensor_mul(d4[:], d2[:], d2[:])
        nc.vector.scalar_tensor_tensor(m4[:], d4[:], -3.0, m4[:], OP.mult, OP.add)

        # v = var + EPS; rstd via ACT sqrt + accurate reciprocal + 2 Newton
        v, s0, r, tn = et("v"), et("s0"), et("r"), et("tn")
        nc.vector.tensor_scalar_add(v[:], featv[:, 1], EPS)
        nc.scalar.activation(s0[:], v[:], A.Sqrt, bias=zero_b[:], scale=1.0)
        nc.vector.reciprocal(r[:], v[:])
        nc.vector.tensor_mul(tn[:], s0[:], r[:])                    # ~ v^-1/2
        tsq, w, u = et("tsq"), et("w"), et("u")
        for _ in range(2):  # Newton: t = t*(1.5 - 0.5*v*t^2)
            nc.vector.tensor_mul(tsq[:], tn[:], tn[:])
            nc.vector.tensor_mul(w[:], v[:], tsq[:])
            nc.vector.scalar_tensor_tensor(u[:], w[:], -0.5, tn[:], OP.mult, OP.mult)
            nc.vector.scalar_tensor_tensor(tn[:], tn[:], 1.5, u[:], OP.mult, OP.add)
        inv3, r2 = et("inv3"), et("r2")
        nc.vector.tensor_mul(inv3[:], tn[:], tn[:])
        nc.vector.tensor_mul(inv3[:], inv3[:], tn[:])               # v^-1.5
        nc.vector.tensor_mul(featv[:, 2], m3[:], inv3[:])           # skew
        nc.vector.tensor_mul(r2[:], r[:], r[:])                     # v^-2
        nc.vector.tensor_mul(featv[:, 3], m4[:], r2[:])             # kurt

        out_v = out.rearrange("b (f h p) -> h b p f", f=4, h=2, p=P)
        feat_v = feat[:].rearrange("p (h b f) -> p h b f", h=2, b=B_LOC, f=4)
        for h in range(2):
            for b in range(B_LOC):
                nc.sync.dma_start(out=out_v[h, b], in_=feat_v[:, h, b])

    nc.compile()
    return nc


def _get(p_val: float, repeat: int = 1):
    key = (p_val, repeat)
    if key not in _CACHE:
        _CACHE[key] = _build(p_val, repeat)
    return _CACHE[key]


def run_sharded(x, p, trace=False, repeat=1, **kw):
    """Run the SPMD kernel on 8 cores. Returns (out [B,4C], BassKernelResults)."""
    from concourse.bass_utils import run_bass_kernel_spmd

    x = np.ascontiguousarray(np.asarray(x, dtype=np.float32))
    assert x.shape == (B, T, C), x.shape
    p_val = float(np.asarray(p).reshape(-1)[0])
    nc = _get(p_val, repeat)
    in_maps = [{"x": x[i * B_LOC:(i + 1) * B_LOC]} for i in range(N_CORES)]
    res = run_bass_kernel_spmd(nc, in_maps, core_ids=list(range(N_CORES)),
                               trace=trace, **kw)
    outp = np.concatenate([r["out"] for r in res.results], axis=0)
    return outp, res


def kernel(x, p):
    return run_sharded(x, p)[0]
